# revision 10
# baseline (speedup 1.0000x reference)
"""Trainium2 Bass kernel for nn_Attention_73718818669284.

Reference computation (per batch b of 2, C=128 channels, N=4096 spatial):
    q = Wq x, k = Wk x, v = Wv x           (1x1 conv == channel matmul)
    w = softmax(q^T k, axis=-1)            ([N, N] attention)
    h = Wo (v w^T)
    y = x + h
    out = SiLU(GroupNorm8(y) * gamma + beta)

Sharding: 8 cores = 2 batches x 4 column-slices of N (1024 each). Each
core receives xb pre-rotated so its slice is always columns 0:1024; the
m-chunk iteration order is a rotation, which softmax/PV sums are
invariant to. GroupNorm statistics are combined across the 4 cores of a
batch with a tiny AllReduce.

Per-core algorithm (transposed-score layout):
    A^T = Wq^T Wk                      (one 128x128 matmul)
    R   = A Xs = Wk^T Wq Xs            ([128, 1024], folds q-projection)
    S^T chunk j = X_j^T R              (f32r matmuls, scores transposed)
    P^T = exp(S^T - 62.5)              (bf16; global shift cancels)
    V^T chunk j = X_j^T Wv^T           (same stationary as scores -> no
                                        separate V matmuls or transposes)
    h_un = sum_j V_j P^T_j             (bf16 matmuls, f32 PSUM accum)
    rowsum = ones^T rsacc              (rsacc accumulated on Pool engine)
    y = x + (Wo h_un) * (1/rowsum)     (normalize commutes with Wo;
                                        reciprocal_approx_fast on DVE)
    stats AllReduce; fused GroupNorm epilogue; SiLU.
"""

import numpy as np

import concourse.bass as bass
import concourse.tile as tile
from concourse import bacc, mybir
from concourse.bass_utils import run_bass_kernel_spmd

F32 = mybir.dt.float32
F32R = mybir.dt.float32r
BF16 = mybir.dt.bfloat16
AF = mybir.ActivationFunctionType
ALU = mybir.AluOpType

P = 128          # channels / partitions
N = 4096         # spatial size (16*16*16)
NS = 1024        # per-core slice of N
NB = N // P      # 32 m-chunks
NCORES = 8
NGROUPS = 8
EPS = 1e-5
CNT = (P // NGROUPS) * N       # elements per group per batch = 16 * 4096
NPARAM = 6 * P + 2     # wq | wk | wvT | woT | G | ones | gamma | beta


def _build_nc():
    nc = bacc.Bacc("TRN2", target_bir_lowering=False, debug=False,
                   num_devices=NCORES)
    # Inputs are declared float32r (same 4-byte layout as f32): the
    # walrus verifier requires every producer feeding an f32r matmul to
    # be f32r itself, and DMA-ing directly into f32r tiles avoids a
    # whole prologue of DVE rounding casts. The PE rounds internally.
    xb = nc.declare_dram_parameter("xb", [P, N], F32R, isOutput=False)
    params = nc.declare_dram_parameter("params", [P, NPARAM], F32R,
                                       isOutput=False)
    out = nc.declare_dram_parameter("out", [P, NS], F32, isOutput=True)
    with tile.TileContext(nc) as tc:
        _emit(nc, tc, xb, params, out)
    nc.compile()
    return nc


def _emit(nc, tc, xb, params, out):
    HALF = NS // 2
    with (
        tc.tile_pool(name="pp", bufs=1) as pp,
        tc.tile_pool(name="ptp", bufs=4) as ptp,
        tc.tile_pool(name="dp", bufs=1, space="DRAM") as dp,
    ):
        # ---- warm-up collective: wakes the CC cores and pulls the
        # runtime init barrier early, overlapping the prologue ----
        warm = pp.tile([1, 2], F32)
        nc.vector.memset(warm[:], 0.0)
        dumc_in = dp.tile([1, 2], F32)
        dumc_out = dp.tile([1, 2], F32)
        nc.sync.dma_start(out=dumc_in[:], in_=warm[:])
        nc.gpsimd.collective_compute(
            "AllReduce", ALU.add,
            replica_groups=[[0, 1, 2, 3], [4, 5, 6, 7]],
            ins=[dumc_in.opt()], outs=[dumc_out.opt()],
        )

        # ---------------- loads (two HWDGE rings in parallel) -----------
        pa_sb = pp.tile([P, NPARAM], F32R)
        nc.scalar.dma_start(out=pa_sb[:], in_=params[:])
        xb_sb = pp.tile([P, N], F32R)
        xb_bf = pp.tile([P, N], BF16)
        for i in range(8):
            nc.sync.dma_start(out=xb_sb[:, i * 512:(i + 1) * 512],
                              in_=xb[:, i * 512:(i + 1) * 512])
            # bf16 shadow of x for the V^T matmuls: f32r matmuls pay
            # 4 cycles/row below 256 free dim; bf16 runs 1 cycle/row.
            nc.vector.tensor_copy(xb_bf[:, i * 512:(i + 1) * 512],
                                  xb_sb[:, i * 512:(i + 1) * 512])

        wq = pa_sb[:, 0:128]
        wk = pa_sb[:, 128:256]
        wvT = pa_sb[:, 256:384]
        woT = pa_sb[:, 384:512]
        gmat = pa_sb[:, 512:640].bitcast(F32)   # group-average matrix
        onesM = pa_sb[:, 640:768]
        gamma_sb = pa_sb[:, 768:769].bitcast(F32)
        beta_sb = pa_sb[:, 769:770].bitcast(F32)

        # Global exp shift: cancels exactly in softmax. Centers the
        # log-rowsum range inside the exp table's clean window.
        shift = pp.tile([P, 1], F32)
        nc.vector.memset(shift[:], -62.5)
        eps_t = pp.tile([P, 1], F32)
        nc.vector.memset(eps_t[:], EPS)

        rsacc_v = pp.tile([P, NS], F32)
        rsacc_p = pp.tile([P, NS], F32)
        rsr = pp.tile([P, NS], F32R)
        rsp = pp.tile([P, NS], F32R)
        vt_sb = pp.tile([P, NB, P], F32R)
        wvT_bf = pp.tile([P, P], BF16)
        nc.vector.tensor_copy(wvT_bf[:], wvT)
        with (
            tc.tile_pool(name="stp", bufs=2, space="PSUM") as stp,
            tc.tile_pool(name="vtp", bufs=2, space="PSUM") as vtp,
            tc.tile_pool(name="acc", bufs=1, space="PSUM") as acc,
        ):
            h_ps = acc.tile([P, NS], F32, tag="h")

            # A^T = Wq^T Wk  -> R = A Xs = Wk^T Wq Xs
            at_ps = stp.tile([P, P], F32, tag="st", name="at_ps")
            nc.tensor.matmul(at_ps[:], wq, wk, start=True, stop=True)
            at_sb = pp.tile([P, P], F32R)
            nc.vector.tensor_copy(at_sb[:], at_ps[:])
            r_ps = stp.tile([P, NS], F32, tag="st", name="r_ps")
            r_r = pp.tile([P, NS], F32R)
            for h in range(2):
                sl = slice(h * 512, (h + 1) * 512)
                nc.tensor.matmul(r_ps[:, sl], at_sb[:], xb_sb[:, sl],
                                 start=True, stop=True)
                nc.vector.tensor_copy(r_r[:, sl], r_ps[:, sl])

            def emit_vgroup(g):
                # V^T chunks 4g..4g+3 directly: vt_j = X_j^T Wv^T
                vt_ps = vtp.tile([P, 4, P], F32, tag="vt", name=f"vt_ps{g}")
                for t in range(4):
                    jj = 4 * g + t
                    nc.tensor.matmul(vt_ps[:, t, :],
                                     xb_bf[:, jj * P:(jj + 1) * P],
                                     wvT_bf[:], start=True, stop=True)
                nc.vector.tensor_copy(vt_sb[:, 4 * g:4 * g + 4, :], vt_ps[:])

            def consume(jj, ptj):
                first = jj == 0
                last = jj == NB - 1
                lhs = vt_sb[:, jj, :]
                nc.tensor.matmul(h_ps[:, 0:512], lhs, ptj[:, 0:512],
                                 start=first, stop=last)
                nc.tensor.matmul(h_ps[:, 512:NS], lhs, ptj[:, 512:NS],
                                 start=first, stop=last)

            def rs_add(jj, ptj):
                # rowsum partials: two independent accumulators, odd
                # chunks on DVE, even on Pool (Pool is slower per op but
                # otherwise idle); the ones-matmul folds both. The last
                # add on each engine writes the f32r fold input directly
                # (fused cast keeps f32 accumulation precision).
                eng, acc_t, last = ((nc.gpsimd, rsacc_p, rsp) if jj % 2 == 0
                                    else (nc.vector, rsacc_v, rsr))
                if jj < 2:
                    eng.tensor_copy(acc_t[:], ptj[:].bitcast(F32))
                elif jj >= NB - 2:
                    eng.tensor_add(last[:], acc_t[:], ptj[:].bitcast(F32))
                else:
                    eng.tensor_add(acc_t[:], acc_t[:], ptj[:].bitcast(F32))

            vg_at = {2 + 4 * g: g for g in range(8)}   # j -> V^T group
            pts = []
            for j in range(NB):
                if j in vg_at:
                    emit_vgroup(vg_at[j])
                st_ps = stp.tile([P, NS], F32, tag="st", name=f"st_ps{j}")
                lhs = xb_sb[:, j * P:(j + 1) * P]
                nc.tensor.matmul(st_ps[:, 0:512], lhs, r_r[:, 0:512],
                                 start=True, stop=True)
                nc.tensor.matmul(st_ps[:, 512:NS], lhs, r_r[:, 512:NS],
                                 start=True, stop=True)
                pt = ptp.tile([P, NS], F32R, tag="pt", name=f"pt{j}")
                nc.scalar.activation(pt[:], st_ps[:], AF.Exp, bias=shift[:])
                pts.append(pt)
                if j >= 2:
                    consume(j - 2, pts[j - 2])
                if j >= 3:
                    rs_add(j - 3, pts[j - 3])
            for jj in (NB - 2, NB - 1):
                consume(jj, pts[jj])
            for jj in (NB - 3, NB - 2, NB - 1):
                rs_add(jj, pts[jj])

            # ---- unnormalized output projection + rowsum reciprocal ----
            # y = x + (Wo h_un) * (1/rowsum): the columnwise normalize
            # commutes with the channel matmul, so Wo runs on the PE as
            # soon as h_un is done, overlapping the rowsum fold.
            hr = pp.tile([P, NS], F32R)
            ab_ps = acc.tile([P, NS], F32, tag="h", name="ab_ps")
            rb_ps = stp.tile([P, NS], F32, tag="st", name="rb_ps")
            y_halves = []
            st4 = pp.tile([P, 4], F32)
            sq_scr = pp.tile([P, 512], F32)
            for h in range(2):
                sl = slice(h * 512, (h + 1) * 512)
                nc.vector.tensor_copy(hr[:, sl], h_ps[:, sl])
                nc.tensor.matmul(ab_ps[:, sl], woT, hr[:, sl],
                                 start=True, stop=True)
                # rowsum broadcast-fold: rb[p, n] = sum_m rsacc[m, n],
                # accumulating both engine partials in PSUM
                nc.tensor.matmul(rb_ps[:, sl], onesM[:], rsr[:, sl],
                                 start=True, stop=False)
                nc.tensor.matmul(rb_ps[:, sl], onesM[:], rsp[:, sl],
                                 start=False, stop=True)
                rbinv = pp.tile([P, 512], F32, name=f"rbinv{h}")
                nc.vector.reciprocal_approx_fast(out=rbinv[:], in_=rb_ps[:, sl])
                t_sb = pp.tile([P, 512], F32, name=f"t{h}")
                nc.vector.tensor_mul(t_sb[:], ab_ps[:, sl], rbinv[:])
                y_sb = pp.tile([P, 512], F32, name=f"y{h}")
                nc.vector.scalar_tensor_tensor(
                    out=y_sb[:], in0=t_sb[:], scalar=1.0,
                    in1=xb_sb[:, sl].bitcast(F32),
                    op0=ALU.mult, op1=ALU.add, accum_out=st4[:, h:h + 1])
                nc.scalar.activation(sq_scr[:], y_sb[:], AF.Square,
                                     accum_out=st4[:, 2 + h:3 + h])
                y_halves.append(y_sb)

            st2 = pp.tile([P, 2], F32)
            nc.vector.tensor_add(st2[:, 0:1], st4[:, 0:1], st4[:, 1:2])
            nc.vector.tensor_add(st2[:, 1:2], st4[:, 2:3], st4[:, 3:4])

            # AllReduce within each batch's 4 cores
            d_st1 = dp.tile([P, 2], F32)
            d_st2 = dp.tile([P, 2], F32)
            nc.sync.dma_start(out=d_st1[:], in_=st2[:])
            nc.gpsimd.collective_compute(
                "AllReduce", ALU.add,
                replica_groups=[[0, 1, 2, 3], [4, 5, 6, 7]],
                ins=[d_st1.opt()], outs=[d_st2.opt()],
            )
            ast_sb = pp.tile([P, 2], F32)
            nc.sync.dma_start(out=ast_sb[:], in_=d_st2[:])

            # fold+broadcast group stats in one matmul with the
            # group-average matrix (includes the 1/CNT scale):
            # pc[c, :] = [mean, E[y^2]] of c's group.
            pc_ps = vtp.tile([P, 2], F32, tag="vt", name="pc_ps")
            nc.tensor.matmul(pc_ps[:], gmat, ast_sb[:], start=True, stop=True)
            pc_sb = pp.tile([P, 2], F32)
            nc.vector.tensor_copy(pc_sb[:], pc_ps[:])
            msq = pp.tile([P, 1], F32)
            nc.vector.tensor_mul(msq[:], pc_sb[:, 0:1], pc_sb[:, 0:1])
            var = pp.tile([P, 1], F32)
            nc.vector.tensor_sub(var[:], pc_sb[:, 1:2], msq[:])
            sd = pp.tile([P, 1], F32)
            nc.scalar.activation(sd[:], var[:], AF.Sqrt, bias=eps_t[:])
            rstd = pp.tile([P, 1], F32)
            nc.vector.reciprocal_approx_fast(out=rstd[:], in_=sd[:])
            # z = (y - mean) * rstd * gamma + beta  ==  y * s1 + s2
            s1 = pp.tile([P, 1], F32)
            nc.vector.tensor_mul(s1[:], rstd[:], gamma_sb)
            ms1 = pp.tile([P, 1], F32)
            nc.vector.tensor_mul(ms1[:], pc_sb[:, 0:1], s1[:])
            s2 = pp.tile([P, 1], F32)
            nc.vector.tensor_sub(s2[:], beta_sb, ms1[:])

            for h in range(2):
                sl = slice(h * 512, (h + 1) * 512)
                z_sb = pp.tile([P, 512], F32, name=f"z{h}")
                nc.vector.tensor_scalar(out=z_sb[:], in0=y_halves[h][:],
                                        scalar1=s1[:], scalar2=s2[:],
                                        op0=ALU.mult, op1=ALU.add)
                o_sb = pp.tile([P, 512], F32, name=f"o{h}")
                nc.scalar.activation(o_sb[:], z_sb[:], AF.Silu)
                nc.sync.dma_start(out=out[:, sl], in_=o_sb[:])


_NC_CACHE = None


def _get_nc():
    global _NC_CACHE
    if _NC_CACHE is None:
        _NC_CACHE = _build_nc()
    return _NC_CACHE


def make_in_maps(x, Wq, Wk, Wv, Wo, gamma, beta):
    x = np.asarray(x, dtype=np.float32)
    B, C = x.shape[0], x.shape[1]
    xf = np.ascontiguousarray(x.reshape(B, C, -1))
    Wq = np.asarray(Wq, dtype=np.float32)
    Wk = np.asarray(Wk, dtype=np.float32)
    WvT = np.asarray(Wv, dtype=np.float32).T
    WoT = np.asarray(Wo, dtype=np.float32).T
    g = np.asarray(gamma, dtype=np.float32).reshape(P, 1)
    b = np.asarray(beta, dtype=np.float32).reshape(P, 1)
    grp = np.arange(P) // (P // NGROUPS)
    gmat = (grp[:, None] == grp[None, :]).astype(np.float32) / CNT
    ones = np.ones((P, P), dtype=np.float32)
    pa = np.ascontiguousarray(
        np.concatenate([Wq, Wk, WvT, WoT, gmat, ones, g, b], axis=1))
    assert pa.shape == (P, NPARAM)

    in_maps = []
    for core in range(NCORES):
        bi, s = core // 4, core % 4
        xrot = np.ascontiguousarray(np.roll(xf[bi], -NS * s, axis=1))
        in_maps.append({"xb": xrot, "params": pa})
    return in_maps


def assemble(results, spatial=(16, 16, 16)):
    y = np.empty((2, P, N), dtype=np.float32)
    for core in range(NCORES):
        bi, s = core // 4, core % 4
        y[bi][:, s * NS:(s + 1) * NS] = results[core]["out"]
    return y.reshape(2, P, *spatial)


def kernel(x, Wq, Wk, Wv, Wo, gamma, beta):
    nc = _get_nc()
    in_maps = make_in_maps(x, Wq, Wk, Wv, Wo, gamma, beta)
    res = run_bass_kernel_spmd(nc, in_maps, list(range(NCORES)))
    return assemble(res.results, spatial=tuple(np.asarray(x).shape[2:]))


# revision 11
# speedup vs baseline: 1.0608x; 1.0608x over previous
"""Trainium2 Bass kernel for nn_Attention_73718818669284.

Reference computation (per batch b of 2, C=128 channels, N=4096 spatial):
    q = Wq x, k = Wk x, v = Wv x           (1x1 conv == channel matmul)
    w = softmax(q^T k, axis=-1)            ([N, N] attention)
    h = Wo (v w^T)
    y = x + h
    out = SiLU(GroupNorm8(y) * gamma + beta)

Sharding: 8 cores = 2 batches x 4 column-slices of N (1024 each). Each
core receives xb pre-rotated so its slice is always columns 0:1024; the
m-chunk iteration order is a rotation, which softmax/PV sums are
invariant to. GroupNorm statistics are combined across the 4 cores of a
batch with a tiny AllReduce.

Per-core algorithm (transposed-score layout):
    A^T = Wq^T Wk                      (one 128x128 matmul)
    R   = A Xs = Wk^T Wq Xs            ([128, 1024], folds q-projection)
    S^T chunk j = X_j^T R              (f32r matmuls, scores transposed)
    P^T = exp(S^T - 62.5)              (bf16; global shift cancels)
    V^T chunk j = X_j^T Wv^T           (same stationary as scores -> no
                                        separate V matmuls or transposes)
    h_un = sum_j V_j P^T_j             (bf16 matmuls, f32 PSUM accum)
    rowsum = ones^T rsacc              (rsacc accumulated on Pool engine)
    y = x + (Wo h_un) * (1/rowsum)     (normalize commutes with Wo;
                                        reciprocal_approx_fast on DVE)
    stats AllReduce; fused GroupNorm epilogue; SiLU.
"""

import numpy as np

import concourse.bass as bass
import concourse.tile as tile
from concourse import bacc, mybir
from concourse.bass_utils import run_bass_kernel_spmd

F32 = mybir.dt.float32
F32R = mybir.dt.float32r
BF16 = mybir.dt.bfloat16
AF = mybir.ActivationFunctionType
ALU = mybir.AluOpType

P = 128          # channels / partitions
N = 4096         # spatial size (16*16*16)
NS = 1024        # per-core slice of N
NB = N // P      # 32 m-chunks
NCORES = 8
NGROUPS = 8
EPS = 1e-5
CNT = (P // NGROUPS) * N       # elements per group per batch = 16 * 4096
NPARAM = 6 * P + 2     # wq | wk | wvT | woT | G | ones | gamma | beta


def _build_nc():
    nc = bacc.Bacc("TRN2", target_bir_lowering=False, debug=False,
                   num_devices=NCORES)
    # Inputs are declared float32r (same 4-byte layout as f32): the
    # walrus verifier requires every producer feeding an f32r matmul to
    # be f32r itself, and DMA-ing directly into f32r tiles avoids a
    # whole prologue of DVE rounding casts. The PE rounds internally.
    xb = nc.declare_dram_parameter("xb", [P, N], F32R, isOutput=False)
    params = nc.declare_dram_parameter("params", [P, NPARAM], F32R,
                                       isOutput=False)
    out = nc.declare_dram_parameter("out", [P, NS], F32, isOutput=True)
    with tile.TileContext(nc) as tc:
        _emit(nc, tc, xb, params, out)
    nc.compile()
    return nc


def _emit(nc, tc, xb, params, out):
    HALF = NS // 2
    with (
        tc.tile_pool(name="pp", bufs=1) as pp,
        tc.tile_pool(name="ptp", bufs=4) as ptp,
        tc.tile_pool(name="dp", bufs=1, space="DRAM") as dp,
    ):
        # ---- warm-up collective: wakes the CC cores and pulls the
        # runtime init barrier early, overlapping the prologue ----
        warm = pp.tile([1, 2], F32)
        nc.vector.memset(warm[:], 0.0)
        dumc_in = dp.tile([1, 2], F32)
        dumc_out = dp.tile([1, 2], F32)
        nc.sync.dma_start(out=dumc_in[:], in_=warm[:])
        nc.gpsimd.collective_compute(
            "AllReduce", ALU.add,
            replica_groups=[[0, 1, 2, 3], [4, 5, 6, 7]],
            ins=[dumc_in.opt()], outs=[dumc_out.opt()],
        )

        # ---------------- loads (two HWDGE rings in parallel) -----------
        pa_sb = pp.tile([P, NPARAM], F32R)
        nc.scalar.dma_start(out=pa_sb[:], in_=params[:])
        xb_sb = pp.tile([P, N], F32R)
        xb_bf = pp.tile([P, N], BF16)
        for i in range(8):
            nc.sync.dma_start(out=xb_sb[:, i * 512:(i + 1) * 512],
                              in_=xb[:, i * 512:(i + 1) * 512])
            # bf16 shadow of x for the V^T matmuls: f32r matmuls pay
            # 4 cycles/row below 256 free dim; bf16 runs 1 cycle/row.
            nc.vector.tensor_copy(xb_bf[:, i * 512:(i + 1) * 512],
                                  xb_sb[:, i * 512:(i + 1) * 512])

        wq = pa_sb[:, 0:128]
        wk = pa_sb[:, 128:256]
        wvT = pa_sb[:, 256:384]
        woT = pa_sb[:, 384:512]
        gmat = pa_sb[:, 512:640].bitcast(F32)   # group-average matrix
        onesM = pa_sb[:, 640:768]
        gamma_sb = pa_sb[:, 768:769].bitcast(F32)
        beta_sb = pa_sb[:, 769:770].bitcast(F32)

        # Global exp shift: cancels exactly in softmax. Centers the
        # log-rowsum range inside the exp table's clean window.
        shift = pp.tile([P, 1], F32)
        nc.vector.memset(shift[:], -62.5)
        eps_t = pp.tile([P, 1], F32)
        nc.vector.memset(eps_t[:], EPS)

        vt_sb = pp.tile([P, NB, P], BF16)
        wvT_bf = pp.tile([P, P], BF16)
        nc.vector.tensor_copy(wvT_bf[:], wvT)
        ones_bf = pp.tile([P, P], BF16)
        nc.vector.tensor_copy(ones_bf[:], onesM[:])
        with (
            tc.tile_pool(name="stp", bufs=1, space="PSUM") as stp,
            tc.tile_pool(name="vtp", bufs=2, space="PSUM") as vtp,
            tc.tile_pool(name="acc", bufs=1, space="PSUM") as acc,
        ):
            h_ps = acc.tile([P, NS], F32, tag="h")
            rb_ps = acc.tile([P, NS], F32, tag="rb")

            # A^T = Wq^T Wk  -> R = A Xs = Wk^T Wq Xs
            at_ps = stp.tile([P, P], F32, tag="st", name="at_ps")
            nc.tensor.matmul(at_ps[:], wq, wk, start=True, stop=True)
            at_sb = pp.tile([P, P], F32R)
            nc.vector.tensor_copy(at_sb[:], at_ps[:])
            r_ps = stp.tile([P, NS], F32, tag="st", name="r_ps")
            r_r = pp.tile([P, NS], F32R)
            for h in range(2):
                sl = slice(h * 512, (h + 1) * 512)
                nc.tensor.matmul(r_ps[:, sl], at_sb[:], xb_sb[:, sl],
                                 start=True, stop=True)
                nc.vector.tensor_copy(r_r[:, sl], r_ps[:, sl])

            def emit_vgroup(g):
                # V^T chunks 4g..4g+3 directly: vt_j = X_j^T Wv^T
                vt_ps = vtp.tile([P, 4, P], F32, tag="vt", name=f"vt_ps{g}")
                for t in range(4):
                    jj = 4 * g + t
                    nc.tensor.matmul(vt_ps[:, t, :],
                                     xb_bf[:, jj * P:(jj + 1) * P],
                                     wvT_bf[:], start=True, stop=True)
                nc.vector.tensor_copy(vt_sb[:, 4 * g:4 * g + 4, :], vt_ps[:])

            def consume(jj, ptj):
                # PV accumulation plus the rowsum broadcast-fold, both as
                # PSUM accumulations on the PE: rb[p, n] += sum_m pt[m, n].
                # This keeps the big elementwise rowsum adds off DVE/Pool,
                # whose SBUF traffic was the loop bottleneck.
                first = jj == 0
                last = jj == NB - 1
                lhs = vt_sb[:, jj, :]
                nc.tensor.matmul(h_ps[:, 0:512], lhs, ptj[:, 0:512],
                                 start=first, stop=last)
                nc.tensor.matmul(h_ps[:, 512:NS], lhs, ptj[:, 512:NS],
                                 start=first, stop=last)
                nc.tensor.matmul(rb_ps[:, 0:512], ones_bf[:], ptj[:, 0:512],
                                 start=first, stop=last)
                nc.tensor.matmul(rb_ps[:, 512:NS], ones_bf[:], ptj[:, 512:NS],
                                 start=first, stop=last)

            vg_at = {1 + 4 * g: g for g in range(8)}   # j -> V^T group
            pts = []
            for j in range(NB):
                st_ps = stp.tile([P, NS], F32, tag="st", name=f"st_ps{j}")
                lhs = xb_sb[:, j * P:(j + 1) * P]
                nc.tensor.matmul(st_ps[:, 0:512], lhs, r_r[:, 0:512],
                                 start=True, stop=True)
                nc.tensor.matmul(st_ps[:, 512:NS], lhs, r_r[:, 512:NS],
                                 start=True, stop=True)
                pt = ptp.tile([P, NS], BF16, tag="pt", name=f"pt{j}")
                nc.scalar.activation(pt[:], st_ps[:], AF.Exp, bias=shift[:])
                pts.append(pt)
                if j >= 2:
                    consume(j - 2, pts[j - 2])
                if j in vg_at:
                    emit_vgroup(vg_at[j])
            for jj in (NB - 2, NB - 1):
                consume(jj, pts[jj])

            # ---- unnormalized output projection + rowsum reciprocal ----
            # y = x + (Wo h_un) * (1/rowsum): the columnwise normalize
            # commutes with the channel matmul, so Wo runs on the PE as
            # soon as h_un is done, overlapping the rowsum fold.
            hr = pp.tile([P, NS], F32R)
            ab_ps = acc.tile([P, NS], F32, tag="h", name="ab_ps")
            y_halves = []
            st4 = pp.tile([P, 4], F32)
            sq_scr = pp.tile([P, 512], F32)
            for h in range(2):
                sl = slice(h * 512, (h + 1) * 512)
                nc.vector.tensor_copy(hr[:, sl], h_ps[:, sl])
                nc.tensor.matmul(ab_ps[:, sl], woT, hr[:, sl],
                                 start=True, stop=True)
                rbinv = pp.tile([P, 512], F32, name=f"rbinv{h}")
                nc.vector.reciprocal_approx_fast(out=rbinv[:], in_=rb_ps[:, sl])
                t_sb = pp.tile([P, 512], F32, name=f"t{h}")
                nc.vector.tensor_mul(t_sb[:], ab_ps[:, sl], rbinv[:])
                y_sb = pp.tile([P, 512], F32, name=f"y{h}")
                nc.vector.scalar_tensor_tensor(
                    out=y_sb[:], in0=t_sb[:], scalar=1.0,
                    in1=xb_sb[:, sl].bitcast(F32),
                    op0=ALU.mult, op1=ALU.add, accum_out=st4[:, h:h + 1])
                nc.scalar.activation(sq_scr[:], y_sb[:], AF.Square,
                                     accum_out=st4[:, 2 + h:3 + h])
                y_halves.append(y_sb)

            st2 = pp.tile([P, 2], F32)
            nc.vector.tensor_add(st2[:, 0:1], st4[:, 0:1], st4[:, 1:2])
            nc.vector.tensor_add(st2[:, 1:2], st4[:, 2:3], st4[:, 3:4])

            # AllReduce within each batch's 4 cores
            d_st1 = dp.tile([P, 2], F32)
            d_st2 = dp.tile([P, 2], F32)
            nc.sync.dma_start(out=d_st1[:], in_=st2[:])
            nc.gpsimd.collective_compute(
                "AllReduce", ALU.add,
                replica_groups=[[0, 1, 2, 3], [4, 5, 6, 7]],
                ins=[d_st1.opt()], outs=[d_st2.opt()],
            )
            ast_sb = pp.tile([P, 2], F32)
            nc.sync.dma_start(out=ast_sb[:], in_=d_st2[:])

            # fold+broadcast group stats in one matmul with the
            # group-average matrix (includes the 1/CNT scale):
            # pc[c, :] = [mean, E[y^2]] of c's group.
            pc_ps = vtp.tile([P, 2], F32, tag="vt", name="pc_ps")
            nc.tensor.matmul(pc_ps[:], gmat, ast_sb[:], start=True, stop=True)
            pc_sb = pp.tile([P, 2], F32)
            nc.vector.tensor_copy(pc_sb[:], pc_ps[:])
            msq = pp.tile([P, 1], F32)
            nc.vector.tensor_mul(msq[:], pc_sb[:, 0:1], pc_sb[:, 0:1])
            var = pp.tile([P, 1], F32)
            nc.vector.tensor_sub(var[:], pc_sb[:, 1:2], msq[:])
            sd = pp.tile([P, 1], F32)
            nc.scalar.activation(sd[:], var[:], AF.Sqrt, bias=eps_t[:])
            rstd = pp.tile([P, 1], F32)
            nc.vector.reciprocal_approx_fast(out=rstd[:], in_=sd[:])
            # z = (y - mean) * rstd * gamma + beta  ==  y * s1 + s2
            s1 = pp.tile([P, 1], F32)
            nc.vector.tensor_mul(s1[:], rstd[:], gamma_sb)
            ms1 = pp.tile([P, 1], F32)
            nc.vector.tensor_mul(ms1[:], pc_sb[:, 0:1], s1[:])
            s2 = pp.tile([P, 1], F32)
            nc.vector.tensor_sub(s2[:], beta_sb, ms1[:])

            for h in range(2):
                sl = slice(h * 512, (h + 1) * 512)
                z_sb = pp.tile([P, 512], F32, name=f"z{h}")
                nc.vector.tensor_scalar(out=z_sb[:], in0=y_halves[h][:],
                                        scalar1=s1[:], scalar2=s2[:],
                                        op0=ALU.mult, op1=ALU.add)
                o_sb = pp.tile([P, 512], F32, name=f"o{h}")
                nc.scalar.activation(o_sb[:], z_sb[:], AF.Silu)
                nc.sync.dma_start(out=out[:, sl], in_=o_sb[:])


_NC_CACHE = None


def _get_nc():
    global _NC_CACHE
    if _NC_CACHE is None:
        _NC_CACHE = _build_nc()
    return _NC_CACHE


def make_in_maps(x, Wq, Wk, Wv, Wo, gamma, beta):
    x = np.asarray(x, dtype=np.float32)
    B, C = x.shape[0], x.shape[1]
    xf = np.ascontiguousarray(x.reshape(B, C, -1))
    Wq = np.asarray(Wq, dtype=np.float32)
    Wk = np.asarray(Wk, dtype=np.float32)
    WvT = np.asarray(Wv, dtype=np.float32).T
    WoT = np.asarray(Wo, dtype=np.float32).T
    g = np.asarray(gamma, dtype=np.float32).reshape(P, 1)
    b = np.asarray(beta, dtype=np.float32).reshape(P, 1)
    grp = np.arange(P) // (P // NGROUPS)
    gmat = (grp[:, None] == grp[None, :]).astype(np.float32) / CNT
    ones = np.ones((P, P), dtype=np.float32)
    pa = np.ascontiguousarray(
        np.concatenate([Wq, Wk, WvT, WoT, gmat, ones, g, b], axis=1))
    assert pa.shape == (P, NPARAM)

    in_maps = []
    for core in range(NCORES):
        bi, s = core // 4, core % 4
        xrot = np.ascontiguousarray(np.roll(xf[bi], -NS * s, axis=1))
        in_maps.append({"xb": xrot, "params": pa})
    return in_maps


def assemble(results, spatial=(16, 16, 16)):
    y = np.empty((2, P, N), dtype=np.float32)
    for core in range(NCORES):
        bi, s = core // 4, core % 4
        y[bi][:, s * NS:(s + 1) * NS] = results[core]["out"]
    return y.reshape(2, P, *spatial)


def kernel(x, Wq, Wk, Wv, Wo, gamma, beta):
    nc = _get_nc()
    in_maps = make_in_maps(x, Wq, Wk, Wv, Wo, gamma, beta)
    res = run_bass_kernel_spmd(nc, in_maps, list(range(NCORES)))
    return assemble(res.results, spatial=tuple(np.asarray(x).shape[2:]))


# revision 12
# speedup vs baseline: 1.1674x; 1.1004x over previous
"""Trainium2 Bass kernel for nn_Attention_73718818669284.

Reference computation (per batch b of 2, C=128 channels, N=4096 spatial):
    q = Wq x, k = Wk x, v = Wv x           (1x1 conv == channel matmul)
    w = softmax(q^T k, axis=-1)            ([N, N] attention)
    h = Wo (v w^T)
    y = x + h
    out = SiLU(GroupNorm8(y) * gamma + beta)

Sharding: 8 cores = 2 batches x 4 column-slices of N (1024 each). Each
core receives xb pre-rotated so its slice is always columns 0:1024; the
m-chunk iteration order is a rotation, which softmax/PV sums are
invariant to. GroupNorm statistics are combined across the 4 cores of a
batch with a tiny AllReduce.

Per-core algorithm (transposed-score layout):
    A^T = Wq^T Wk                      (one 128x128 matmul)
    R   = A Xs = Wk^T Wq Xs            ([128, 1024], folds q-projection)
    S^T chunk j = X_j^T R              (f32r matmuls, scores transposed)
    P^T = exp(S^T - 62.5)              (bf16; global shift cancels)
    V^T chunk j = X_j^T Wv^T           (same stationary as scores -> no
                                        separate V matmuls or transposes)
    h_un = sum_j V_j P^T_j             (bf16 matmuls, f32 PSUM accum)
    rowsum = ones^T rsacc              (rsacc accumulated on Pool engine)
    y = x + (Wo h_un) * (1/rowsum)     (normalize commutes with Wo;
                                        reciprocal_approx_fast on DVE)
    stats AllReduce; fused GroupNorm epilogue; SiLU.
"""

import numpy as np

import concourse.bass as bass
import concourse.tile as tile
from concourse import bacc, mybir
from concourse.bass_utils import run_bass_kernel_spmd

F32 = mybir.dt.float32
F32R = mybir.dt.float32r
BF16 = mybir.dt.bfloat16
AF = mybir.ActivationFunctionType
ALU = mybir.AluOpType

P = 128          # channels / partitions
N = 4096         # spatial size (16*16*16)
NS = 1024        # per-core slice of N
NB = N // P      # 32 m-chunks
NCORES = 8
NGROUPS = 8
EPS = 1e-5
CNT = (P // NGROUPS) * N       # elements per group per batch = 16 * 4096
NPARAM = 6 * P + 2     # wq | wk | wvT | woT | G | ones | gamma | beta


def _build_nc():
    nc = bacc.Bacc("TRN2", target_bir_lowering=False, debug=False,
                   num_devices=NCORES)
    # Inputs are declared float32r (same 4-byte layout as f32): the
    # walrus verifier requires every producer feeding an f32r matmul to
    # be f32r itself, and DMA-ing directly into f32r tiles avoids a
    # whole prologue of DVE rounding casts. The PE rounds internally.
    xb = nc.declare_dram_parameter("xb", [P, N], F32R, isOutput=False)
    params = nc.declare_dram_parameter("params", [P, NPARAM], F32R,
                                       isOutput=False)
    out = nc.declare_dram_parameter("out", [P, NS], F32, isOutput=True)
    with tile.TileContext(nc) as tc:
        _emit(nc, tc, xb, params, out)
    nc.compile()
    return nc


def _emit(nc, tc, xb, params, out):
    HALF = NS // 2
    with (
        tc.tile_pool(name="pp", bufs=1) as pp,
        tc.tile_pool(name="ptp", bufs=4) as ptp,
        tc.tile_pool(name="dp", bufs=1, space="DRAM") as dp,
    ):
        # ---- warm-up collective: wakes the CC cores and pulls the
        # runtime init barrier early, overlapping the prologue ----
        warm = pp.tile([1, 2], F32)
        nc.vector.memset(warm[:], 0.0)
        dumc_in = dp.tile([1, 2], F32)
        dumc_out = dp.tile([1, 2], F32)
        nc.sync.dma_start(out=dumc_in[:], in_=warm[:])
        nc.gpsimd.collective_compute(
            "AllReduce", ALU.add,
            replica_groups=[[0, 1, 2, 3], [4, 5, 6, 7]],
            ins=[dumc_in.opt()], outs=[dumc_out.opt()],
        )

        # ---------------- loads (two HWDGE rings in parallel) -----------
        pa_sb = pp.tile([P, NPARAM], F32R)
        nc.scalar.dma_start(out=pa_sb[:], in_=params[:])
        xb_sb = pp.tile([P, N], F32R)
        xb_bf = pp.tile([P, N], BF16)
        for i in range(8):
            nc.sync.dma_start(out=xb_sb[:, i * 512:(i + 1) * 512],
                              in_=xb[:, i * 512:(i + 1) * 512])
            # bf16 shadow of x for the V^T matmuls: f32r matmuls pay
            # 4 cycles/row below 256 free dim; bf16 runs 1 cycle/row.
            nc.vector.tensor_copy(xb_bf[:, i * 512:(i + 1) * 512],
                                  xb_sb[:, i * 512:(i + 1) * 512])

        wq = pa_sb[:, 0:128]
        wk = pa_sb[:, 128:256]
        wvT = pa_sb[:, 256:384]
        woT = pa_sb[:, 384:512]
        gmat = pa_sb[:, 512:640].bitcast(F32)   # group-average matrix
        onesM = pa_sb[:, 640:768]
        gamma_sb = pa_sb[:, 768:769].bitcast(F32)
        beta_sb = pa_sb[:, 769:770].bitcast(F32)

        # Global exp shift: cancels exactly in softmax. Centers the
        # log-rowsum range inside the exp table's clean window.
        shift = pp.tile([P, 1], F32)
        nc.vector.memset(shift[:], -62.5)
        eps_t = pp.tile([P, 1], F32)
        nc.vector.memset(eps_t[:], EPS)

        vt_sb = pp.tile([P, NB, P], BF16)
        wvT_bf = pp.tile([P, P], BF16)
        nc.vector.tensor_copy(wvT_bf[:], wvT)
        rsacc_v = pp.tile([P, NS], F32)
        rsacc_p = pp.tile([P, NS], F32)
        rsr = pp.tile([P, NS], F32R)
        rsp = pp.tile([P, NS], F32R)
        with (
            tc.tile_pool(name="stp", bufs=2, space="PSUM") as stp,
            tc.tile_pool(name="vtp", bufs=2, space="PSUM") as vtp,
            tc.tile_pool(name="acc", bufs=1, space="PSUM") as acc,
        ):
            h_ps = acc.tile([P, NS], F32, tag="h")

            # A^T = Wq^T Wk  -> R = A Xs = Wk^T Wq Xs
            at_ps = stp.tile([P, P], F32, tag="st", name="at_ps")
            nc.tensor.matmul(at_ps[:], wq, wk, start=True, stop=True)
            at_sb = pp.tile([P, P], F32R)
            nc.vector.tensor_copy(at_sb[:], at_ps[:])
            r_ps = stp.tile([P, NS], F32, tag="st", name="r_ps")
            r_r = pp.tile([P, NS], F32R)
            for h in range(2):
                sl = slice(h * 512, (h + 1) * 512)
                nc.tensor.matmul(r_ps[:, sl], at_sb[:], xb_sb[:, sl],
                                 start=True, stop=True)
                nc.vector.tensor_copy(r_r[:, sl], r_ps[:, sl])

            def emit_vgroup(g):
                # V^T chunks 4g..4g+3 directly: vt_j = X_j^T Wv^T
                vt_ps = vtp.tile([P, 4, P], F32, tag="vt", name=f"vt_ps{g}")
                for t in range(4):
                    jj = 4 * g + t
                    nc.tensor.matmul(vt_ps[:, t, :],
                                     xb_bf[:, jj * P:(jj + 1) * P],
                                     wvT_bf[:], start=True, stop=True)
                nc.vector.tensor_copy(vt_sb[:, 4 * g:4 * g + 4, :], vt_ps[:])

            def consume(jj, ptj):
                first = jj == 0
                last = jj == NB - 1
                lhs = vt_sb[:, jj, :]
                nc.tensor.matmul(h_ps[:, 0:512], lhs, ptj[:, 0:512],
                                 start=first, stop=last)
                nc.tensor.matmul(h_ps[:, 512:NS], lhs, ptj[:, 512:NS],
                                 start=first, stop=last)

            def rs_add(jj, ptj):
                # rowsum partials: 3/4 of chunks on DVE, 1/4 on the slower
                # Pool engine, independent accumulators. The last add per
                # engine writes the f32r fold input directly (fused cast;
                # accumulation itself stays f32).
                eng, acc_t, fin = ((nc.gpsimd, rsacc_p, rsp) if jj % 4 == 0
                                   else (nc.vector, rsacc_v, rsr))
                if jj <= 1:
                    eng.tensor_copy(acc_t[:], ptj[:])
                elif jj >= NB - 4:
                    last = jj == 28 if jj % 4 == 0 else jj == NB - 1
                    if last:
                        eng.tensor_add(fin[:], acc_t[:], ptj[:])
                    else:
                        eng.tensor_add(acc_t[:], acc_t[:], ptj[:])
                else:
                    eng.tensor_add(acc_t[:], acc_t[:], ptj[:])

            vg_at = {1 + 4 * g: g for g in range(8)}   # j -> V^T group
            pts = []
            for j in range(NB):
                st_ps = stp.tile([P, NS], F32, tag="st", name=f"st_ps{j}")
                lhs = xb_sb[:, j * P:(j + 1) * P]
                nc.tensor.matmul(st_ps[:, 0:512], lhs, r_r[:, 0:512],
                                 start=True, stop=True)
                nc.tensor.matmul(st_ps[:, 512:NS], lhs, r_r[:, 512:NS],
                                 start=True, stop=True)
                pt = ptp.tile([P, NS], BF16, tag="pt", name=f"pt{j}")
                nc.scalar.activation(pt[:], st_ps[:], AF.Exp, bias=shift[:])
                pts.append(pt)
                if j >= 2:
                    consume(j - 2, pts[j - 2])
                if j >= 3:
                    rs_add(j - 3, pts[j - 3])
                if j in vg_at:
                    emit_vgroup(vg_at[j])
            for jj in (NB - 2, NB - 1):
                consume(jj, pts[jj])
            for jj in (NB - 3, NB - 2, NB - 1):
                rs_add(jj, pts[jj])

            # ---- unnormalized output projection + rowsum reciprocal ----
            # y = x + (Wo h_un) * (1/rowsum): the columnwise normalize
            # commutes with the channel matmul, so Wo runs on the PE as
            # soon as h_un is done, overlapping the rowsum fold.
            hr = pp.tile([P, NS], F32R)
            ab_ps = acc.tile([P, NS], F32, tag="h", name="ab_ps")
            rb_ps = stp.tile([P, NS], F32, tag="st", name="rb_ps")
            y_halves = []
            st4 = pp.tile([P, 4], F32)
            sq_scr = pp.tile([P, 512], F32)
            for h in range(2):
                sl = slice(h * 512, (h + 1) * 512)
                nc.scalar.activation(hr[:, sl], h_ps[:, sl], AF.Copy)
                nc.tensor.matmul(ab_ps[:, sl], woT, hr[:, sl],
                                 start=True, stop=True)
                # rowsum broadcast-fold of both engine partials
                nc.tensor.matmul(rb_ps[:, sl], onesM[:], rsr[:, sl],
                                 start=True, stop=False)
                nc.tensor.matmul(rb_ps[:, sl], onesM[:], rsp[:, sl],
                                 start=False, stop=True)
                rbinv = pp.tile([P, 512], F32, name=f"rbinv{h}")
                nc.vector.reciprocal_approx_fast(out=rbinv[:], in_=rb_ps[:, sl])
                t_sb = pp.tile([P, 512], F32, name=f"t{h}")
                nc.vector.tensor_mul(t_sb[:], ab_ps[:, sl], rbinv[:])
                y_sb = pp.tile([P, 512], F32, name=f"y{h}")
                nc.vector.scalar_tensor_tensor(
                    out=y_sb[:], in0=t_sb[:], scalar=1.0,
                    in1=xb_sb[:, sl].bitcast(F32),
                    op0=ALU.mult, op1=ALU.add, accum_out=st4[:, h:h + 1])
                nc.scalar.activation(sq_scr[:], y_sb[:], AF.Square,
                                     accum_out=st4[:, 2 + h:3 + h])
                y_halves.append(y_sb)

            st2 = pp.tile([P, 2], F32)
            nc.vector.tensor_add(st2[:, 0:1], st4[:, 0:1], st4[:, 1:2])
            nc.vector.tensor_add(st2[:, 1:2], st4[:, 2:3], st4[:, 3:4])

            # AllReduce within each batch's 4 cores
            d_st1 = dp.tile([P, 2], F32)
            d_st2 = dp.tile([P, 2], F32)
            nc.sync.dma_start(out=d_st1[:], in_=st2[:])
            nc.gpsimd.collective_compute(
                "AllReduce", ALU.add,
                replica_groups=[[0, 1, 2, 3], [4, 5, 6, 7]],
                ins=[d_st1.opt()], outs=[d_st2.opt()],
            )
            ast_sb = pp.tile([P, 2], F32)
            nc.sync.dma_start(out=ast_sb[:], in_=d_st2[:])

            # fold+broadcast group stats in one matmul with the
            # group-average matrix (includes the 1/CNT scale):
            # pc[c, :] = [mean, E[y^2]] of c's group.
            pc_ps = vtp.tile([P, 2], F32, tag="vt", name="pc_ps")
            nc.tensor.matmul(pc_ps[:], gmat, ast_sb[:], start=True, stop=True)
            pc_sb = pp.tile([P, 2], F32)
            nc.vector.tensor_copy(pc_sb[:], pc_ps[:])
            msq = pp.tile([P, 1], F32)
            nc.vector.tensor_mul(msq[:], pc_sb[:, 0:1], pc_sb[:, 0:1])
            var = pp.tile([P, 1], F32)
            nc.vector.tensor_sub(var[:], pc_sb[:, 1:2], msq[:])
            sd = pp.tile([P, 1], F32)
            nc.scalar.activation(sd[:], var[:], AF.Sqrt, bias=eps_t[:])
            rstd = pp.tile([P, 1], F32)
            nc.vector.reciprocal_approx_fast(out=rstd[:], in_=sd[:])
            # z = (y - mean) * rstd * gamma + beta  ==  y * s1 + s2
            s1 = pp.tile([P, 1], F32)
            nc.vector.tensor_mul(s1[:], rstd[:], gamma_sb)
            ms1 = pp.tile([P, 1], F32)
            nc.vector.tensor_mul(ms1[:], pc_sb[:, 0:1], s1[:])
            s2 = pp.tile([P, 1], F32)
            nc.vector.tensor_sub(s2[:], beta_sb, ms1[:])

            for h in range(2):
                sl = slice(h * 512, (h + 1) * 512)
                z_sb = pp.tile([P, 512], F32, name=f"z{h}")
                nc.vector.tensor_scalar(out=z_sb[:], in0=y_halves[h][:],
                                        scalar1=s1[:], scalar2=s2[:],
                                        op0=ALU.mult, op1=ALU.add)
                o_sb = pp.tile([P, 512], F32, name=f"o{h}")
                nc.scalar.activation(o_sb[:], z_sb[:], AF.Silu)
                nc.sync.dma_start(out=out[:, sl], in_=o_sb[:])


_NC_CACHE = None


def _get_nc():
    global _NC_CACHE
    if _NC_CACHE is None:
        _NC_CACHE = _build_nc()
    return _NC_CACHE


def make_in_maps(x, Wq, Wk, Wv, Wo, gamma, beta):
    x = np.asarray(x, dtype=np.float32)
    B, C = x.shape[0], x.shape[1]
    xf = np.ascontiguousarray(x.reshape(B, C, -1))
    Wq = np.asarray(Wq, dtype=np.float32)
    Wk = np.asarray(Wk, dtype=np.float32)
    WvT = np.asarray(Wv, dtype=np.float32).T
    WoT = np.asarray(Wo, dtype=np.float32).T
    g = np.asarray(gamma, dtype=np.float32).reshape(P, 1)
    b = np.asarray(beta, dtype=np.float32).reshape(P, 1)
    grp = np.arange(P) // (P // NGROUPS)
    gmat = (grp[:, None] == grp[None, :]).astype(np.float32) / CNT
    ones = np.ones((P, P), dtype=np.float32)
    pa = np.ascontiguousarray(
        np.concatenate([Wq, Wk, WvT, WoT, gmat, ones, g, b], axis=1))
    assert pa.shape == (P, NPARAM)

    in_maps = []
    for core in range(NCORES):
        bi, s = core // 4, core % 4
        xrot = np.ascontiguousarray(np.roll(xf[bi], -NS * s, axis=1))
        in_maps.append({"xb": xrot, "params": pa})
    return in_maps


def assemble(results, spatial=(16, 16, 16)):
    y = np.empty((2, P, N), dtype=np.float32)
    for core in range(NCORES):
        bi, s = core // 4, core % 4
        y[bi][:, s * NS:(s + 1) * NS] = results[core]["out"]
    return y.reshape(2, P, *spatial)


def kernel(x, Wq, Wk, Wv, Wo, gamma, beta):
    nc = _get_nc()
    in_maps = make_in_maps(x, Wq, Wk, Wv, Wo, gamma, beta)
    res = run_bass_kernel_spmd(nc, in_maps, list(range(NCORES)))
    return assemble(res.results, spatial=tuple(np.asarray(x).shape[2:]))


# revision 13
# speedup vs baseline: 1.1743x; 1.0059x over previous
"""Trainium2 Bass kernel for nn_Attention_73718818669284.

Reference computation (per batch b of 2, C=128 channels, N=4096 spatial):
    q = Wq x, k = Wk x, v = Wv x           (1x1 conv == channel matmul)
    w = softmax(q^T k, axis=-1)            ([N, N] attention)
    h = Wo (v w^T)
    y = x + h
    out = SiLU(GroupNorm8(y) * gamma + beta)

Sharding: 8 cores = 2 batches x 4 column-slices of N (1024 each). Each
core receives xb pre-rotated so its slice is always columns 0:1024; the
m-chunk iteration order is a rotation, which softmax/PV sums are
invariant to. GroupNorm statistics are combined across the 4 cores of a
batch with a tiny AllReduce.

Per-core algorithm (transposed-score layout):
    A^T = Wq^T Wk                      (one 128x128 matmul)
    R   = A Xs = Wk^T Wq Xs            ([128, 1024], folds q-projection)
    S^T chunk j = X_j^T R              (f32r matmuls, scores transposed)
    P^T = exp(S^T - 62.5)              (bf16; global shift cancels)
    V^T chunk j = X_j^T Wv^T           (same stationary as scores -> no
                                        separate V matmuls or transposes)
    h_un = sum_j V_j P^T_j             (bf16 matmuls, f32 PSUM accum)
    rowsum = ones^T rsacc              (rsacc accumulated on Pool engine)
    y = x + (Wo h_un) * (1/rowsum)     (normalize commutes with Wo;
                                        reciprocal_approx_fast on DVE)
    stats AllReduce; fused GroupNorm epilogue; SiLU.
"""

import numpy as np

import concourse.bass as bass
import concourse.tile as tile
from concourse import bacc, mybir
from concourse.bass_utils import run_bass_kernel_spmd

F32 = mybir.dt.float32
F32R = mybir.dt.float32r
BF16 = mybir.dt.bfloat16
AF = mybir.ActivationFunctionType
ALU = mybir.AluOpType

P = 128          # channels / partitions
N = 4096         # spatial size (16*16*16)
NS = 1024        # per-core slice of N
NB = N // P      # 32 m-chunks
NCORES = 8
NGROUPS = 8
EPS = 1e-5
CNT = (P // NGROUPS) * N       # elements per group per batch = 16 * 4096
NPARAM = 6 * P + 2     # wq | wk | wvT | woT | G | ones | gamma | beta


def _build_nc():
    nc = bacc.Bacc("TRN2", target_bir_lowering=False, debug=False,
                   num_devices=NCORES)
    # Inputs are declared float32r (same 4-byte layout as f32): the
    # walrus verifier requires every producer feeding an f32r matmul to
    # be f32r itself, and DMA-ing directly into f32r tiles avoids a
    # whole prologue of DVE rounding casts. The PE rounds internally.
    xb = nc.declare_dram_parameter("xb", [P, N], F32R, isOutput=False)
    params = nc.declare_dram_parameter("params", [P, NPARAM], F32R,
                                       isOutput=False)
    out = nc.declare_dram_parameter("out", [P, NS], F32, isOutput=True)
    with tile.TileContext(nc) as tc:
        _emit(nc, tc, xb, params, out)
    nc.compile()
    return nc


def _emit(nc, tc, xb, params, out):
    HALF = NS // 2
    with (
        tc.tile_pool(name="pp", bufs=1) as pp,
        tc.tile_pool(name="ptp", bufs=6) as ptp,
        tc.tile_pool(name="dp", bufs=1, space="DRAM") as dp,
    ):
        # ---------------- loads (two HWDGE rings in parallel) -----------
        pa_sb = pp.tile([P, NPARAM], F32R)
        nc.scalar.dma_start(out=pa_sb[:], in_=params[:])
        xb_sb = pp.tile([P, N], F32R)
        xb_bf = pp.tile([P, N], BF16)
        for i in range(8):
            nc.sync.dma_start(out=xb_sb[:, i * 512:(i + 1) * 512],
                              in_=xb[:, i * 512:(i + 1) * 512])
            # bf16 shadow of x for the V^T matmuls: f32r matmuls pay
            # 4 cycles/row below 256 free dim; bf16 runs 1 cycle/row.
            nc.vector.tensor_copy(xb_bf[:, i * 512:(i + 1) * 512],
                                  xb_sb[:, i * 512:(i + 1) * 512])

        # PE warm-up: dummy matmuls with no data deps keep the PE busy
        # from NEFF start, so the HAM clock gate reaches 2.4 GHz before
        # the real chain (which waits ~13us on the params/x DMAs) begins.
        zb = pp.tile([P, 512], BF16)
        nc.vector.memset(zb[:], 0.0)
        warm_ps = stp_warm = None

        wq = pa_sb[:, 0:128]
        wk = pa_sb[:, 128:256]
        wvT = pa_sb[:, 256:384]
        woT = pa_sb[:, 384:512]
        gmat = pa_sb[:, 512:640].bitcast(F32)   # group-average matrix
        onesM = pa_sb[:, 640:768]
        gamma_sb = pa_sb[:, 768:769].bitcast(F32)
        beta_sb = pa_sb[:, 769:770].bitcast(F32)

        # Global exp shift: cancels exactly in softmax. Centers the
        # log-rowsum range inside the exp table's clean window.
        shift = pp.tile([P, 1], F32)
        nc.vector.memset(shift[:], -62.5)
        eps_t = pp.tile([P, 1], F32)
        nc.vector.memset(eps_t[:], EPS)

        vt_sb = pp.tile([P, NB, P], BF16)
        wvT_bf = pp.tile([P, P], BF16)
        nc.vector.tensor_copy(wvT_bf[:], wvT)
        rsacc_v = pp.tile([P, NS], F32)
        rsacc_p = pp.tile([P, NS], F32)
        rsr = pp.tile([P, NS], F32R)
        rsp = pp.tile([P, NS], F32R)
        with (
            tc.tile_pool(name="stp", bufs=2, space="PSUM") as stp,
            tc.tile_pool(name="vtp", bufs=2, space="PSUM") as vtp,
            tc.tile_pool(name="acc", bufs=1, space="PSUM") as acc,
        ):
            h_ps = acc.tile([P, NS], F32, tag="h")

            for w in range(6):
                wp = stp.tile([P, 512], F32, tag="st", name=f"warm{w}")
                nc.tensor.matmul(wp[:], zb[:, 0:128], zb[:],
                                 start=True, stop=True)

            # A^T = Wq^T Wk  -> R = A Xs = Wk^T Wq Xs
            at_ps = stp.tile([P, P], F32, tag="st", name="at_ps")
            nc.tensor.matmul(at_ps[:], wq, wk, start=True, stop=True)
            at_sb = pp.tile([P, P], F32R)
            nc.vector.tensor_copy(at_sb[:], at_ps[:])
            r_ps = stp.tile([P, NS], F32, tag="st", name="r_ps")
            r_r = pp.tile([P, NS], F32R)
            for h in range(2):
                sl = slice(h * 512, (h + 1) * 512)
                nc.tensor.matmul(r_ps[:, sl], at_sb[:], xb_sb[:, sl],
                                 start=True, stop=True)
                nc.vector.tensor_copy(r_r[:, sl], r_ps[:, sl])

            def emit_vgroup(g):
                # V^T chunks 4g..4g+3 directly: vt_j = X_j^T Wv^T
                vt_ps = vtp.tile([P, 4, P], F32, tag="vt", name=f"vt_ps{g}")
                for t in range(4):
                    jj = 4 * g + t
                    nc.tensor.matmul(vt_ps[:, t, :],
                                     xb_bf[:, jj * P:(jj + 1) * P],
                                     wvT_bf[:], start=True, stop=True)
                nc.vector.tensor_copy(vt_sb[:, 4 * g:4 * g + 4, :], vt_ps[:])

            def consume(jj, ptj):
                first = jj == 0
                last = jj == NB - 1
                lhs = vt_sb[:, jj, :]
                nc.tensor.matmul(h_ps[:, 0:512], lhs, ptj[:, 0:512],
                                 start=first, stop=last)
                nc.tensor.matmul(h_ps[:, 512:NS], lhs, ptj[:, 512:NS],
                                 start=first, stop=last)

            POOL_RS = set(range(0, NB - 2, 4)) | {NB - 2}
            def rs_add(jj, ptj):
                # rowsum partials: ~3/4 of chunks on DVE, 1/4 on the
                # slower Pool engine, independent accumulators; the
                # ones-matmul folds both. The last add per engine writes
                # the f32r fold input directly (fused cast; accumulation
                # itself stays f32).
                pool = jj in POOL_RS
                eng, acc_t, fin = ((nc.gpsimd, rsacc_p, rsp) if pool
                                   else (nc.vector, rsacc_v, rsr))
                first = jj == 0 if pool else jj == 1
                last = jj == NB - 2 if pool else jj == NB - 1
                if first:
                    eng.tensor_copy(acc_t[:], ptj[:])
                elif last:
                    eng.tensor_add(fin[:], acc_t[:], ptj[:])
                else:
                    eng.tensor_add(acc_t[:], acc_t[:], ptj[:])

            vg_at = {1 + 4 * g: g for g in range(8)}   # j -> V^T group
            pts = []
            for j in range(NB):
                st_ps = stp.tile([P, NS], F32, tag="st", name=f"st_ps{j}")
                lhs = xb_sb[:, j * P:(j + 1) * P]
                nc.tensor.matmul(st_ps[:, 0:512], lhs, r_r[:, 0:512],
                                 start=True, stop=True)
                nc.tensor.matmul(st_ps[:, 512:NS], lhs, r_r[:, 512:NS],
                                 start=True, stop=True)
                pt = ptp.tile([P, NS], BF16, tag="pt", name=f"pt{j}")
                nc.scalar.activation(pt[:], st_ps[:], AF.Exp, bias=shift[:])
                pts.append(pt)
                if j >= 2:
                    consume(j - 2, pts[j - 2])
                if j >= 3:
                    rs_add(j - 3, pts[j - 3])
                if j in vg_at:
                    emit_vgroup(vg_at[j])
            for jj in (NB - 2, NB - 1):
                consume(jj, pts[jj])
            for jj in (NB - 3, NB - 2, NB - 1):
                rs_add(jj, pts[jj])

            # ---- unnormalized output projection + rowsum reciprocal ----
            # y = x + (Wo h_un) * (1/rowsum): the columnwise normalize
            # commutes with the channel matmul, so Wo runs on the PE as
            # soon as h_un is done, overlapping the rowsum fold.
            hr = pp.tile([P, NS], F32R)
            ab_ps = acc.tile([P, NS], F32, tag="h", name="ab_ps")
            rb_ps = stp.tile([P, NS], F32, tag="st", name="rb_ps")
            y_halves = []
            st4 = pp.tile([P, 4], F32)
            sq_scr = pp.tile([P, 512], F32)
            for h in range(2):
                sl = slice(h * 512, (h + 1) * 512)
                nc.scalar.activation(hr[:, sl], h_ps[:, sl], AF.Copy)
                nc.tensor.matmul(ab_ps[:, sl], woT, hr[:, sl],
                                 start=True, stop=True)
                # rowsum broadcast-fold of both engine partials
                nc.tensor.matmul(rb_ps[:, sl], onesM[:], rsr[:, sl],
                                 start=True, stop=False)
                nc.tensor.matmul(rb_ps[:, sl], onesM[:], rsp[:, sl],
                                 start=False, stop=True)
                rbinv = pp.tile([P, 512], F32, name=f"rbinv{h}")
                nc.vector.reciprocal_approx_fast(out=rbinv[:], in_=rb_ps[:, sl])
                t_sb = pp.tile([P, 512], F32, name=f"t{h}")
                nc.vector.tensor_mul(t_sb[:], ab_ps[:, sl], rbinv[:])
                y_sb = pp.tile([P, 512], F32, name=f"y{h}")
                nc.vector.scalar_tensor_tensor(
                    out=y_sb[:], in0=t_sb[:], scalar=1.0,
                    in1=xb_sb[:, sl].bitcast(F32),
                    op0=ALU.mult, op1=ALU.add, accum_out=st4[:, h:h + 1])
                nc.scalar.activation(sq_scr[:], y_sb[:], AF.Square,
                                     accum_out=st4[:, 2 + h:3 + h])
                y_halves.append(y_sb)

            st2 = pp.tile([P, 2], F32)
            nc.vector.tensor_add(st2[:, 0:1], st4[:, 0:1], st4[:, 1:2])
            nc.vector.tensor_add(st2[:, 1:2], st4[:, 2:3], st4[:, 3:4])

            # AllReduce within each batch's 4 cores
            d_st1 = dp.tile([P, 2], F32)
            d_st2 = dp.tile([P, 2], F32)
            nc.scalar.dma_start(out=d_st1[:], in_=st2[:])
            nc.gpsimd.collective_compute(
                "AllReduce", ALU.add,
                replica_groups=[[0, 1, 2, 3], [4, 5, 6, 7]],
                ins=[d_st1.opt()], outs=[d_st2.opt()],
            )
            ast_sb = pp.tile([P, 2], F32)
            nc.scalar.dma_start(out=ast_sb[:], in_=d_st2[:])

            # fold+broadcast group stats in one matmul with the
            # group-average matrix (includes the 1/CNT scale):
            # pc[c, :] = [mean, E[y^2]] of c's group.
            pc_ps = vtp.tile([P, 2], F32, tag="vt", name="pc_ps")
            nc.tensor.matmul(pc_ps[:], gmat, ast_sb[:], start=True, stop=True)
            pc_sb = pp.tile([P, 2], F32)
            nc.vector.tensor_copy(pc_sb[:], pc_ps[:])
            # varn = mean^2 - E[y^2]; sd = sqrt(-varn + eps)
            varn = pp.tile([P, 1], F32)
            nc.vector.scalar_tensor_tensor(
                out=varn[:], in0=pc_sb[:, 0:1], scalar=pc_sb[:, 0:1],
                in1=pc_sb[:, 1:2], op0=ALU.mult, op1=ALU.subtract)
            sd = pp.tile([P, 1], F32)
            nc.scalar.activation(sd[:], varn[:], AF.Sqrt, bias=eps_t[:],
                                 scale=-1.0)
            rstd = pp.tile([P, 1], F32)
            nc.vector.reciprocal_approx_fast(out=rstd[:], in_=sd[:])
            # z = (y - mean) * rstd * gamma + beta == y * s1 - s2n
            s1 = pp.tile([P, 1], F32)
            nc.vector.tensor_scalar_mul(s1[:], in0=rstd[:],
                                        scalar1=gamma_sb)
            s2n = pp.tile([P, 1], F32)
            nc.vector.scalar_tensor_tensor(
                out=s2n[:], in0=s1[:], scalar=pc_sb[:, 0:1],
                in1=beta_sb, op0=ALU.mult, op1=ALU.subtract)

            for h in range(2):
                sl = slice(h * 512, (h + 1) * 512)
                z_sb = pp.tile([P, 512], F32, name=f"z{h}")
                nc.vector.tensor_scalar(out=z_sb[:], in0=y_halves[h][:],
                                        scalar1=s1[:], scalar2=s2n[:],
                                        op0=ALU.mult, op1=ALU.subtract)
                o_sb = pp.tile([P, 512], F32, name=f"o{h}")
                nc.scalar.activation(o_sb[:], z_sb[:], AF.Silu)
                nc.sync.dma_start(out=out[:, sl], in_=o_sb[:])


_NC_CACHE = None


def _get_nc():
    global _NC_CACHE
    if _NC_CACHE is None:
        _NC_CACHE = _build_nc()
    return _NC_CACHE


def make_in_maps(x, Wq, Wk, Wv, Wo, gamma, beta):
    x = np.asarray(x, dtype=np.float32)
    B, C = x.shape[0], x.shape[1]
    xf = np.ascontiguousarray(x.reshape(B, C, -1))
    Wq = np.asarray(Wq, dtype=np.float32)
    Wk = np.asarray(Wk, dtype=np.float32)
    WvT = np.asarray(Wv, dtype=np.float32).T
    WoT = np.asarray(Wo, dtype=np.float32).T
    g = np.asarray(gamma, dtype=np.float32).reshape(P, 1)
    b = np.asarray(beta, dtype=np.float32).reshape(P, 1)
    grp = np.arange(P) // (P // NGROUPS)
    gmat = (grp[:, None] == grp[None, :]).astype(np.float32) / CNT
    ones = np.ones((P, P), dtype=np.float32)
    pa = np.ascontiguousarray(
        np.concatenate([Wq, Wk, WvT, WoT, gmat, ones, g, b], axis=1))
    assert pa.shape == (P, NPARAM)

    in_maps = []
    for core in range(NCORES):
        bi, s = core // 4, core % 4
        xrot = np.ascontiguousarray(np.roll(xf[bi], -NS * s, axis=1))
        in_maps.append({"xb": xrot, "params": pa})
    return in_maps


def assemble(results, spatial=(16, 16, 16)):
    y = np.empty((2, P, N), dtype=np.float32)
    for core in range(NCORES):
        bi, s = core // 4, core % 4
        y[bi][:, s * NS:(s + 1) * NS] = results[core]["out"]
    return y.reshape(2, P, *spatial)


def kernel(x, Wq, Wk, Wv, Wo, gamma, beta):
    nc = _get_nc()
    in_maps = make_in_maps(x, Wq, Wk, Wv, Wo, gamma, beta)
    res = run_bass_kernel_spmd(nc, in_maps, list(range(NCORES)))
    return assemble(res.results, spatial=tuple(np.asarray(x).shape[2:]))
